# revision 28
# baseline (speedup 1.0000x reference)
"""Bass/Trainium2 kernel for nn_Encoder (embedding -> BiLSTM -> cross attention -> enhancement).

Sharding: data-parallel over batch, 16 items per core on 8 NeuronCores
(per the data-parallel hint; no collectives needed). Per core the A and B
sequences are stacked into 32 rows and the fwd/bwd LSTM directions run as two
interleaved dependency chains sharing the engines.

Phases per core: (1) input projections x@Wih^T+bias for both directions as
dense f32r matmuls staged to DRAM, (2) the 128-step recurrent scan — per step
and direction 16 h^T@Whh^T f32r matmuls into PSUM, per-bank DVE adds of the
staged xw, per-gate in-place activations in [g,i,f,o] order (so tanh(g)
starts after the first PSUM bank), cell/hidden elementwise, and a PE
transpose producing the next step's stationary h^T, (3) cross-attention:
PE transposes to feature-major, E/E^T f32 matmuls, row softmaxes via
Exp-with-accum, soft alignments as f32r matmuls, and the 4-way enhancement
concat streamed straight to the outputs.

float32r (full-rate fp32 PE mode, ~1e-4 matmul rel err) is used for all
large-N matmuls; elementwise math and the small-N attention logit matmuls
stay fp32.
"""

import numpy as np

V, E, H = 32000, 300, 512
BSZ, T = 128, 128
NCORES = 8
PB = BSZ // NCORES          # 16 batch items per core
RW = 2 * PB                 # 32 stacked rows (A items then B items)
RT = 2 * RW                 # 64 rows in fused fwd+bwd elementwise space
G4 = 4 * H                  # 2048 gate width
H2 = 2 * H                  # 1024 bilstm output width
KCH = [(0, 128), (128, 128), (256, 44)]   # chunks of E=300

_CACHE = {}


def _build(phases=3, scan_T=T, xwp_bufs=2, gp_bufs=2, a3_bufs=2, eps_bufs=2):
    import concourse.mybir as mybir
    import concourse.tile as tile
    from concourse import bacc
    from concourse.masks import make_identity

    F32 = mybir.dt.float32
    F32R = mybir.dt.float32r
    F16 = mybir.dt.float16
    AF = mybir.ActivationFunctionType
    ALU = mybir.AluOpType
    AX = mybir.AxisListType

    nc = bacc.Bacc("TRN2", target_bir_lowering=False, debug=False,
                   num_devices=NCORES)

    xT_d = nc.dram_tensor("xT", [E, RW * T], F32R, kind="ExternalInput")
    wih_d = {d: nc.dram_tensor(f"wihT_{d}", [E, G4], F32R, kind="ExternalInput")
             for d in "fb"}
    whh_d = {d: nc.dram_tensor(f"whhT_{d}", [H, G4], F32R, kind="ExternalInput")
             for d in "fb"}
    bias_d = {d: nc.dram_tensor(f"bias_{d}", [128, G4], F32, kind="ExternalInput")
              for d in "fb"}
    outA_d = nc.dram_tensor("outA", [PB, T, 4 * H2], F32, kind="ExternalOutput")
    outB_d = nc.dram_tensor("outB", [PB, T, 4 * H2], F32, kind="ExternalOutput")

    with tile.TileContext(nc) as tc:
        with tc.tile_pool(name="dram", bufs=1, space="DRAM") as dpool, \
             tc.tile_pool(name="const", bufs=1) as const:
            xw = {d: dpool.tile([RW, T, G4], F16, name=f"xw_{d}") for d in "fb"}
            tm = dpool.tile([RW, T, H2], F32R)
            ident = const.tile([128, 128], F32)
            make_identity(nc, ident[:])
            identr = const.tile([128, 128], F32R)
            nc.vector.tensor_copy(identr[:], ident[:])
            ident16 = const.tile([128, 128], F16)
            nc.vector.tensor_copy(ident16[:], ident[:])

            # ---------------- Phase 1: input projections ----------------
            with tc.tile_pool(name="p1w", bufs=1) as p1w, \
                 tc.tile_pool(name="p1ps", bufs=2, space="PSUM") as p1ps, \
                 tc.tile_pool(name="p1e", bufs=3) as p1e:
                xT_sb = []
                for ki, (ko, ks) in enumerate(KCH):
                    t_ = p1w.tile([ks, RW * T], F32R, tag=f"xT{ki}")
                    nc.sync.dma_start(t_[:], xT_d.ap()[ko:ko + ks, :])
                    xT_sb.append(t_)
                for d in "fb":
                    wih_sb = []
                    for ki, (ko, ks) in enumerate(KCH):
                        t_ = p1w.tile([ks, G4], F32R, tag=f"wih{d}{ki}")
                        nc.sync.dma_start(t_[:], wih_d[d].ap()[ko:ko + ks, :])
                        wih_sb.append(t_)
                    bias_sb = p1w.tile([128, G4], F32, tag=f"bias{d}")
                    nc.sync.dma_start(bias_sb[:], bias_d[d].ap())
                    for rc in range(RW):
                        ps = p1ps.tile([128, G4], F32, tag="pj")
                        for nj in range(4):
                            for ki in range(3):
                                nc.tensor.matmul(
                                    ps[:, nj * 512:(nj + 1) * 512],
                                    xT_sb[ki][:, rc * T:(rc + 1) * T],
                                    wih_sb[ki][:, nj * 512:(nj + 1) * 512],
                                    start=(ki == 0), stop=(ki == 2))
                        ev = p1e.tile([128, G4], F16, tag="ev")
                        nc.vector.tensor_add(ev[:], ps[:], bias_sb[:])
                        nc.sync.dma_start(xw[d][rc, :, :], ev[:])

            # ---------------- Phase 2: bidirectional LSTM scan ----------------
            if phases < 2:
                nc.compile()
                return nc
            with tc.tile_pool(name="wst", bufs=1) as wst, \
                 tc.tile_pool(name="sst", bufs=1) as sst, \
                 tc.tile_pool(name="xwp", bufs=xwp_bufs) as xwp, \
                 tc.tile_pool(name="gp", bufs=gp_bufs) as gp, \
                 tc.tile_pool(name="gps", bufs=1, space="PSUM") as gps_pool, \
                 tc.tile_pool(name="tps", bufs=2, space="PSUM") as tps_pool:
                whh_sb = {}
                for d in "fb":
                    whh_sb[d] = []
                    for kc in range(4):
                        w = wst.tile([128, G4], F32R, tag=f"whh{d}{kc}")
                        nc.sync.dma_start(w[:], whh_d[d].ap()[kc * 128:(kc + 1) * 128, :])
                        whh_sb[d].append(w)
                # hT_d: transposed h state per direction; chunk c in cols [32c:32c+32]
                hT = {d: sst.tile([128, 4 * RW], F32R, name=f"hT_{d}") for d in "fb"}
                c_st = {d: sst.tile([RW, H], F32, name=f"c_st_{d}") for d in "fb"}

                # gates layout (host permuted): [g | i | f | o]
                GG, GI, GF, GO = 0, 1, 2, 3
                for t in range(scan_T):
                    for di, d in enumerate("fb"):
                        tx = t if d == "f" else T - 1 - t
                        xwt = xwp.tile([RW, G4], F16, tag=f"xwt{d}", name=f"xwt{d}")
                        nc.sync.dma_start(xwt[:], xw[d][:, tx, :])
                        sgall = gp.tile([RW, G4], F32, tag=f"sgall{d}",
                                        name=f"sgall{d}")

                        def bank(nj):
                            return slice(nj * H, (nj + 1) * H)

                        if t == 0:
                            # h == 0: gates are just xw + bias (bias folded in xw)
                            for nj in range(4):
                                nc.vector.tensor_copy(sgall[:, bank(nj)],
                                                      xwt[:, bank(nj)])
                        else:
                            gps = gps_pool.tile([RW, G4], F32, tag=f"g{d}",
                                                name=f"gps{d}")
                            for nj in range(4):
                                for kc in range(4):
                                    nc.tensor.matmul(
                                        gps[:, bank(nj)],
                                        hT[d][:, 32 * kc:32 * kc + RW],
                                        whh_sb[d][kc][:, bank(nj)],
                                        start=(kc == 0), stop=(kc == 3))
                                nc.vector.tensor_add(sgall[:, bank(nj)],
                                                     gps[:, bank(nj)],
                                                     xwt[:, bank(nj)])
                        # activations in-place per gate; order [g, i, f, o]
                        nc.scalar.activation(sgall[:, bank(GG)], sgall[:, bank(GG)],
                                             AF.Tanh)
                        nc.scalar.activation(sgall[:, bank(GI)], sgall[:, bank(GI)],
                                             AF.Sigmoid)
                        p_ = gp.tile([RW, H], F32, tag=f"p_{d}", name=f"p_{d}")
                        nc.gpsimd.tensor_mul(p_[:], sgall[:, bank(GI)],
                                             sgall[:, bank(GG)])
                        nc.scalar.activation(sgall[:, bank(GF)], sgall[:, bank(GF)],
                                             AF.Sigmoid)
                        if t == 0:
                            nc.vector.tensor_copy(c_st[d][:], p_[:])
                        else:
                            q_ = gp.tile([RW, H], F32, tag=f"q_{d}", name=f"q_{d}")
                            nc.gpsimd.tensor_mul(q_[:], sgall[:, bank(GF)], c_st[d][:])
                            nc.vector.tensor_add(c_st[d][:], p_[:], q_[:])
                        nc.scalar.activation(sgall[:, bank(GO)], sgall[:, bank(GO)],
                                             AF.Sigmoid)
                        th = gp.tile([RW, H], F32, tag=f"th{d}", name=f"th{d}")
                        nc.scalar.activation(th[:], c_st[d][:], AF.Tanh)
                        h_ = gp.tile([RW, H], F32R, tag=f"h_{d}", name=f"h_{d}")
                        nc.vector.tensor_mul(h_[:], sgall[:, bank(GO)], th[:])
                        tp = gps_pool.tile([128, 4 * RW], F32R, tag=f"g{d}",
                                           name=f"tp{d}")
                        for cc in range(4):
                            nc.tensor.transpose(tp[:, RW * cc:RW * cc + RW],
                                                h_[:, 128 * cc:128 * cc + 128],
                                                identr[0:RW, 0:RW])
                        nc.vector.tensor_copy(hT[d][:], tp[:])
                        lo, hi = (0, H) if d == "f" else (H, H2)
                        nc.sync.dma_start(tm[:, tx, lo:hi], h_[:])
                        nc.sync.dma_start(outA_d.ap()[:, tx, lo:hi],
                                          h_[0:PB, :].bitcast(F32))
                        nc.sync.dma_start(outB_d.ap()[:, tx, lo:hi],
                                          h_[PB:RW, :].bitcast(F32))

            # ---------------- Phase 3: attention + enhancement ----------------
            if phases < 3:
                nc.compile()
                return nc
            with tc.tile_pool(name="a3", bufs=a3_bufs) as a3, \
                 tc.tile_pool(name="a3s", bufs=2) as a3s, \
                 tc.tile_pool(name="eps", bufs=eps_bufs, space="PSUM") as eps_pool, \
                 tc.tile_pool(name="tp3", bufs=2, space="PSUM") as tp3_pool, \
                 tc.tile_pool(name="ops", bufs=2, space="PSUM") as ops_pool:
                for n in range(PB):
                    a_tm = a3.tile([128, H2], F32R, tag="a_tm")
                    nc.sync.dma_start(a_tm[:], tm[n, :, :])
                    b_tm = a3.tile([128, H2], F32R, tag="b_tm")
                    nc.sync.dma_start(b_tm[:], tm[PB + n, :, :])
                    a_fm = a3.tile([128, H2], F32, tag="a_fm")
                    b_fm = a3.tile([128, H2], F32, tag="b_fm")
                    for src, dst in ((a_tm, a_fm), (b_tm, b_fm)):
                        for cc in range(8):
                            tp3 = tp3_pool.tile([128, 128], F32R, tag="tp3")
                            nc.tensor.transpose(tp3[:], src[:, 128 * cc:128 * (cc + 1)],
                                                identr[:])
                            nc.vector.tensor_copy(dst[:, 128 * cc:128 * (cc + 1)],
                                                  tp3[:].bitcast(F32))
                    e_ps = eps_pool.tile([128, 128], F32, tag="e")
                    e2_ps = eps_pool.tile([128, 128], F32, tag="e")
                    for cc in range(8):
                        sl = slice(128 * cc, 128 * (cc + 1))
                        nc.tensor.matmul(e_ps[:], a_fm[:, sl], b_fm[:, sl],
                                         start=(cc == 0), stop=(cc == 7))
                    for cc in range(8):
                        sl = slice(128 * cc, 128 * (cc + 1))
                        nc.tensor.matmul(e2_ps[:], b_fm[:, sl], a_fm[:, sl],
                                         start=(cc == 0), stop=(cc == 7))
                    zs, rs = [], []
                    for eps in (e_ps, e2_ps):
                        m_ = a3s.tile([128, 1], F32, tag="m_")
                        nc.vector.tensor_reduce(m_[:], eps[:], axis=AX.X,
                                                op=ALU.max, negate=True)
                        z_ = a3s.tile([128, 128], F32, tag="z_")
                        s_ = a3s.tile([128, 1], F32, tag="s_")
                        nc.scalar.activation(z_[:], eps[:], AF.Exp, bias=m_[:],
                                             accum_out=s_[:])
                        r_ = a3s.tile([128, 1], F32, tag="r_")
                        nc.vector.reciprocal(r_[:], s_[:])
                        zt_ps = tp3_pool.tile([128, 128], F32, tag="tp3")
                        nc.tensor.transpose(zt_ps[:], z_[:], ident[:])
                        zt = a3s.tile([128, 128], F32R, tag="zt")
                        nc.vector.tensor_copy(zt[:], zt_ps[:])
                        zs.append(zt)
                        rs.append(r_)
                    tilded = []
                    for zt, r_, rhs_tm in ((zs[0], rs[0], b_tm), (zs[1], rs[1], a_tm)):
                        t_ps = ops_pool.tile([128, H2], F32, tag="t_ps")
                        for half in range(2):
                            sl = slice(512 * half, 512 * (half + 1))
                            nc.tensor.matmul(t_ps[:, sl], zt[:], rhs_tm[:, sl],
                                             start=True, stop=True)
                        til = a3.tile([128, H2], F32, tag="til")
                        nc.vector.tensor_scalar_mul(til[:], t_ps[:], r_[:])
                        tilded.append(til)
                    for bar, til, outd in ((a_tm, tilded[0], outA_d),
                                           (b_tm, tilded[1], outB_d)):
                        nc.sync.dma_start(outd.ap()[n, :, H2:2 * H2], til[:])
                        df = a3.tile([128, H2], F32, tag="df")
                        nc.gpsimd.tensor_sub(df[:], bar[:].bitcast(F32), til[:])
                        nc.sync.dma_start(outd.ap()[n, :, 2 * H2:3 * H2], df[:])
                        pr = a3.tile([128, H2], F32, tag="pr")
                        nc.vector.tensor_mul(pr[:], bar[:].bitcast(F32), til[:])
                        nc.sync.dma_start(outd.ap()[n, :, 3 * H2:4 * H2], pr[:])

    nc.compile()
    return nc


def _get_nc():
    if "nc" not in _CACHE:
        _CACHE["nc"] = _build()
    return _CACHE["nc"]


def prep_in_maps(inputs):
    A = np.asarray(inputs["A"])
    B = np.asarray(inputs["B"])
    embed = np.asarray(inputs["embed"], dtype=np.float32)
    # permute pytorch gate order [i,f,g,o] -> [g,i,f,o]
    perm = np.concatenate([np.arange(2 * H, 3 * H), np.arange(0, 2 * H),
                           np.arange(3 * H, 4 * H)])
    wmat, bmat = {}, {}
    for d in "fb":
        suf = "_f" if d == "f" else "_b"
        wihT = np.ascontiguousarray(
            np.asarray(inputs["Wih" + suf], dtype=np.float32)[perm].T)
        whhT = np.ascontiguousarray(
            np.asarray(inputs["Whh" + suf], dtype=np.float32)[perm].T)
        bias = (np.asarray(inputs["bih" + suf], dtype=np.float32)
                + np.asarray(inputs["bhh" + suf], dtype=np.float32))[perm]
        bias_bc = np.ascontiguousarray(
            np.broadcast_to(bias[None, :], (128, G4)), dtype=np.float32)
        wmat[d] = (wihT, whhT)
        bmat[d] = bias_bc

    xa = embed[A]    # [BSZ, T, E]
    xb = embed[B]

    in_maps = []
    for c in range(NCORES):
        sl = slice(PB * c, PB * (c + 1))
        xc = np.concatenate([xa[sl], xb[sl]], axis=0)          # [RW, T, E]
        xT = np.ascontiguousarray(
            xc.transpose(2, 0, 1).reshape(E, RW * T), dtype=np.float32)
        in_maps.append({
            "xT": xT,
            "wihT_f": wmat["f"][0], "whhT_f": wmat["f"][1], "bias_f": bmat["f"],
            "wihT_b": wmat["b"][0], "whhT_b": wmat["b"][1], "bias_b": bmat["b"],
        })
    return in_maps


def kernel(**inputs):
    from concourse.bass_utils import run_bass_kernel_spmd

    in_maps = prep_in_maps(inputs)
    nc = _get_nc()
    res = run_bass_kernel_spmd(nc, in_maps, core_ids=list(range(NCORES)))
    outA = np.concatenate([res.results[c]["outA"] for c in range(NCORES)], axis=0)
    outB = np.concatenate([res.results[c]["outB"] for c in range(NCORES)], axis=0)
    return outA, outB


# ---------------------------------------------------------------------------
# Two-NEFF variant: run1 = proj + one (seq, dir, half-batch) scan per core;
# run2 = batch-sharded attention. Host reshuffles hidden states in between and
# writes the "bar" output quarter directly from run1's results.
B1 = 64  # batch rows per run1 core


def _build_run1():
    import concourse.mybir as mybir
    import concourse.tile as tile
    from concourse import bacc
    from concourse.masks import make_identity

    F32 = mybir.dt.float32
    F32R = mybir.dt.float32r
    F16 = mybir.dt.float16
    AF = mybir.ActivationFunctionType

    nc = bacc.Bacc("TRN2", target_bir_lowering=False, debug=False,
                   num_devices=NCORES)
    xT_d = nc.dram_tensor("xT", [E, B1 * T], F32R, kind="ExternalInput")
    wih_d = nc.dram_tensor("wihT", [E, G4], F32R, kind="ExternalInput")
    whh_d = nc.dram_tensor("whhT", [H, G4], F32R, kind="ExternalInput")
    bias_d = nc.dram_tensor("bias", [128, G4], F32, kind="ExternalInput")
    tm_d = nc.dram_tensor("tm1", [B1, T, H], F32, kind="ExternalOutput")

    with tile.TileContext(nc) as tc:
        with tc.tile_pool(name="dram", bufs=1, space="DRAM") as dpool, \
             tc.tile_pool(name="const", bufs=1) as const:
            xw = dpool.tile([B1, T, G4], F16, name="xw1")
            ident = const.tile([128, 128], F32)
            make_identity(nc, ident[:])
            identr = const.tile([128, 128], F32R)
            nc.vector.tensor_copy(identr[:], ident[:])

            # proj
            with tc.tile_pool(name="p1w", bufs=1) as p1w, \
                 tc.tile_pool(name="p1ps", bufs=2, space="PSUM") as p1ps, \
                 tc.tile_pool(name="p1e", bufs=3) as p1e:
                xT_sb, wih_sb = [], []
                for ki, (ko, ks) in enumerate(KCH):
                    t_ = p1w.tile([ks, B1 * T], F32R, tag=f"xT{ki}", name=f"xT{ki}")
                    nc.sync.dma_start(t_[:], xT_d.ap()[ko:ko + ks, :])
                    xT_sb.append(t_)
                    w_ = p1w.tile([ks, G4], F32R, tag=f"wih{ki}", name=f"wih{ki}")
                    nc.sync.dma_start(w_[:], wih_d.ap()[ko:ko + ks, :])
                    wih_sb.append(w_)
                bias_sb = p1w.tile([128, G4], F32, tag="bias")
                nc.sync.dma_start(bias_sb[:], bias_d.ap())
                for rc in range(B1):
                    ps = p1ps.tile([128, G4], F32, tag="pj")
                    for nj in range(4):
                        for ki in range(3):
                            nc.tensor.matmul(
                                ps[:, nj * 512:(nj + 1) * 512],
                                xT_sb[ki][:, rc * T:(rc + 1) * T],
                                wih_sb[ki][:, nj * 512:(nj + 1) * 512],
                                start=(ki == 0), stop=(ki == 2))
                    ev = p1e.tile([128, G4], F16, tag="ev")
                    nc.vector.tensor_add(ev[:], ps[:], bias_sb[:])
                    nc.sync.dma_start(xw[rc, :, :], ev[:])

            # scan (single direction; bwd cores get host-reversed inputs)
            with tc.tile_pool(name="wst", bufs=1) as wst, \
                 tc.tile_pool(name="sst", bufs=1) as sst, \
                 tc.tile_pool(name="xwp", bufs=3) as xwp, \
                 tc.tile_pool(name="gp", bufs=2) as gp, \
                 tc.tile_pool(name="gps", bufs=1, space="PSUM") as gps_pool:
                whh_sb = []
                for kc in range(4):
                    w = wst.tile([128, G4], F32R, tag=f"whh{kc}", name=f"whh{kc}")
                    nc.sync.dma_start(w[:], whh_d.ap()[kc * 128:(kc + 1) * 128, :])
                    whh_sb.append(w)
                hT = sst.tile([128, 4 * B1], F32R, name="hT1")
                c_st = sst.tile([B1, H], F32, name="c_st1")

                GG, GI, GF, GO = 0, 1, 2, 3
                for t in range(T):
                    xwt = xwp.tile([B1, G4], F16, tag="xwt", name="xwt")
                    nc.sync.dma_start(xwt[:], xw[:, t, :])
                    sgall = gp.tile([B1, G4], F32, tag="sgall", name="sgall")

                    def bank(nj):
                        return slice(nj * H, (nj + 1) * H)

                    if t == 0:
                        for nj in range(4):
                            nc.vector.tensor_copy(sgall[:, bank(nj)],
                                                  xwt[:, bank(nj)])
                    else:
                        gps = gps_pool.tile([B1, G4], F32, tag="g", name="gps1")
                        for nj in range(4):
                            for kc in range(4):
                                nc.tensor.matmul(
                                    gps[:, bank(nj)],
                                    hT[:, B1 * kc:B1 * kc + B1],
                                    whh_sb[kc][:, bank(nj)],
                                    start=(kc == 0), stop=(kc == 3))
                            nc.vector.tensor_add(sgall[:, bank(nj)],
                                                 gps[:, bank(nj)],
                                                 xwt[:, bank(nj)])
                    nc.scalar.activation(sgall[:, bank(GG)], sgall[:, bank(GG)],
                                         AF.Tanh)
                    nc.scalar.activation(sgall[:, bank(GI)], sgall[:, bank(GI)],
                                         AF.Sigmoid)
                    p_ = gp.tile([B1, H], F32, tag="p_", name="p_")
                    nc.gpsimd.tensor_mul(p_[:], sgall[:, bank(GI)],
                                         sgall[:, bank(GG)])
                    nc.scalar.activation(sgall[:, bank(GF)], sgall[:, bank(GF)],
                                         AF.Sigmoid)
                    if t == 0:
                        nc.vector.tensor_copy(c_st[:], p_[:])
                    else:
                        q_ = gp.tile([B1, H], F32, tag="q_", name="q_")
                        nc.gpsimd.tensor_mul(q_[:], sgall[:, bank(GF)], c_st[:])
                        nc.vector.tensor_add(c_st[:], p_[:], q_[:])
                    nc.scalar.activation(sgall[:, bank(GO)], sgall[:, bank(GO)],
                                         AF.Sigmoid)
                    th = gp.tile([B1, H], F32, tag="th", name="th")
                    nc.scalar.activation(th[:], c_st[:], AF.Tanh)
                    h_ = gp.tile([B1, H], F32, tag="h_", name="h_")
                    nc.vector.tensor_mul(h_[:], sgall[:, bank(GO)], th[:])
                    tp = gps_pool.tile([128, 4 * B1], F32R, tag="g", name="tp1")
                    for cc in range(4):
                        nc.tensor.transpose(tp[:, B1 * cc:B1 * cc + B1],
                                            h_[:, 128 * cc:128 * cc + 128]
                                            .bitcast(F32R),
                                            identr[0:B1, 0:B1])
                    nc.vector.tensor_copy(hT[:], tp[:])
                    nc.sync.dma_start(tm_d.ap()[:, t, :], h_[:])
    nc.compile()
    return nc


def _build_run2():
    import concourse.mybir as mybir
    import concourse.tile as tile
    from concourse import bacc
    from concourse.masks import make_identity

    F32 = mybir.dt.float32
    F32R = mybir.dt.float32r
    AF = mybir.ActivationFunctionType
    ALU = mybir.AluOpType
    AX = mybir.AxisListType

    nc = bacc.Bacc("TRN2", target_bir_lowering=False, debug=False,
                   num_devices=NCORES)
    tmA_d = nc.dram_tensor("tmA", [PB, T, H2], F32R, kind="ExternalInput")
    tmB_d = nc.dram_tensor("tmB", [PB, T, H2], F32R, kind="ExternalInput")
    oA_d = nc.dram_tensor("oA", [PB, T, 3 * H2], F32, kind="ExternalOutput")
    oB_d = nc.dram_tensor("oB", [PB, T, 3 * H2], F32, kind="ExternalOutput")

    with tile.TileContext(nc) as tc:
        with tc.tile_pool(name="const", bufs=1) as const, \
             tc.tile_pool(name="a3", bufs=2) as a3, \
             tc.tile_pool(name="a3s", bufs=2) as a3s, \
             tc.tile_pool(name="eps", bufs=2, space="PSUM") as eps_pool, \
             tc.tile_pool(name="tp3", bufs=2, space="PSUM") as tp3_pool, \
             tc.tile_pool(name="ops", bufs=2, space="PSUM") as ops_pool:
            ident = const.tile([128, 128], F32)
            make_identity(nc, ident[:])
            identr = const.tile([128, 128], F32R)
            nc.vector.tensor_copy(identr[:], ident[:])
            for n in range(PB):
                a_tm = a3.tile([128, H2], F32R, tag="a_tm")
                nc.sync.dma_start(a_tm[:], tmA_d.ap()[n, :, :])
                b_tm = a3.tile([128, H2], F32R, tag="b_tm")
                nc.sync.dma_start(b_tm[:], tmB_d.ap()[n, :, :])
                a_fm = a3.tile([128, H2], F32, tag="a_fm")
                b_fm = a3.tile([128, H2], F32, tag="b_fm")
                for src_, dst in ((a_tm, a_fm), (b_tm, b_fm)):
                    for cc in range(8):
                        tp3 = tp3_pool.tile([128, 128], F32R, tag="tp3")
                        nc.tensor.transpose(tp3[:], src_[:, 128 * cc:128 * (cc + 1)],
                                            identr[:])
                        nc.vector.tensor_copy(dst[:, 128 * cc:128 * (cc + 1)],
                                              tp3[:].bitcast(F32))
                e_ps = eps_pool.tile([128, 128], F32, tag="e")
                e2_ps = eps_pool.tile([128, 128], F32, tag="e")
                for cc in range(8):
                    sl = slice(128 * cc, 128 * (cc + 1))
                    nc.tensor.matmul(e_ps[:], a_fm[:, sl], b_fm[:, sl],
                                     start=(cc == 0), stop=(cc == 7))
                for cc in range(8):
                    sl = slice(128 * cc, 128 * (cc + 1))
                    nc.tensor.matmul(e2_ps[:], b_fm[:, sl], a_fm[:, sl],
                                     start=(cc == 0), stop=(cc == 7))
                zs, rs = [], []
                for eps in (e_ps, e2_ps):
                    m_ = a3s.tile([128, 1], F32, tag="m_")
                    nc.vector.tensor_reduce(m_[:], eps[:], axis=AX.X,
                                            op=ALU.max, negate=True)
                    z_ = a3s.tile([128, 128], F32, tag="z_")
                    s_ = a3s.tile([128, 1], F32, tag="s_")
                    nc.scalar.activation(z_[:], eps[:], AF.Exp, bias=m_[:],
                                         accum_out=s_[:])
                    r_ = a3s.tile([128, 1], F32, tag="r_")
                    nc.vector.reciprocal(r_[:], s_[:])
                    zt_ps = tp3_pool.tile([128, 128], F32, tag="tp3")
                    nc.tensor.transpose(zt_ps[:], z_[:], ident[:])
                    zt = a3s.tile([128, 128], F32R, tag="zt")
                    nc.vector.tensor_copy(zt[:], zt_ps[:])
                    zs.append(zt)
                    rs.append(r_)
                tilded = []
                for zt, r_, rhs_tm in ((zs[0], rs[0], b_tm), (zs[1], rs[1], a_tm)):
                    t_ps = ops_pool.tile([128, H2], F32, tag="t_ps")
                    for half in range(2):
                        sl = slice(512 * half, 512 * (half + 1))
                        nc.tensor.matmul(t_ps[:, sl], zt[:], rhs_tm[:, sl],
                                         start=True, stop=True)
                    til = a3.tile([128, H2], F32, tag="til")
                    nc.vector.tensor_scalar_mul(til[:], t_ps[:], r_[:])
                    tilded.append(til)
                for bar, til, outd in ((a_tm, tilded[0], oA_d),
                                       (b_tm, tilded[1], oB_d)):
                    nc.sync.dma_start(outd.ap()[n, :, 0:H2], til[:])
                    df = a3.tile([128, H2], F32, tag="df")
                    nc.gpsimd.tensor_sub(df[:], bar[:].bitcast(F32), til[:])
                    nc.sync.dma_start(outd.ap()[n, :, H2:2 * H2], df[:])
                    pr = a3.tile([128, H2], F32, tag="pr")
                    nc.vector.tensor_mul(pr[:], bar[:].bitcast(F32), til[:])
                    nc.sync.dma_start(outd.ap()[n, :, 2 * H2:3 * H2], pr[:])
    nc.compile()
    return nc


def kernel2(**inputs):
    """Two-NEFF variant: run1 scans, host reshuffle, run2 attention."""
    from concourse.bass_utils import run_bass_kernel_spmd

    A = np.asarray(inputs["A"])
    B = np.asarray(inputs["B"])
    embed = np.asarray(inputs["embed"], dtype=np.float32)
    perm = np.concatenate([np.arange(2 * H, 3 * H), np.arange(0, 2 * H),
                           np.arange(3 * H, 4 * H)])
    wp = {}
    for d in "fb":
        suf = "_f" if d == "f" else "_b"
        wihT = np.ascontiguousarray(
            np.asarray(inputs["Wih" + suf], dtype=np.float32)[perm].T)
        whhT = np.ascontiguousarray(
            np.asarray(inputs["Whh" + suf], dtype=np.float32)[perm].T)
        bias = (np.asarray(inputs["bih" + suf], dtype=np.float32)
                + np.asarray(inputs["bhh" + suf], dtype=np.float32))[perm]
        bias_bc = np.ascontiguousarray(
            np.broadcast_to(bias[None, :], (128, G4)), dtype=np.float32)
        wp[d] = (wihT, whhT, bias_bc)

    x_seq = {0: embed[A], 1: embed[B]}   # [BSZ, T, E]

    # core c: seq = c//4, dir = (c//2)%2 (0=f,1=b), half = c%2
    in_maps1 = []
    meta = []
    for c in range(NCORES):
        seq, dirb, half = c // 4, (c // 2) % 2, c % 2
        d = "fb"[dirb]
        xs = x_seq[seq][B1 * half:B1 * (half + 1)]       # [64, T, E]
        if d == "b":
            xs = xs[:, ::-1, :]                          # reversed time
        xT = np.ascontiguousarray(
            xs.transpose(2, 0, 1).reshape(E, B1 * T), dtype=np.float32)
        wihT, whhT, bias_bc = wp[d]
        in_maps1.append({"xT": xT, "wihT": wihT, "whhT": whhT, "bias": bias_bc})
        meta.append((seq, d, half))

    if "nc1" not in _CACHE:
        _CACHE["nc1"] = _build_run1()
    res1 = run_bass_kernel_spmd(_CACHE["nc1"], in_maps1,
                                core_ids=list(range(NCORES)))

    tm_full = {0: np.empty((BSZ, T, H2), np.float32),
               1: np.empty((BSZ, T, H2), np.float32)}
    for c, (seq, d, half) in enumerate(meta):
        tm1 = res1.results[c]["tm1"]                     # [64, T, 512]
        if d == "b":
            tm1 = tm1[:, ::-1, :]
        lo = 0 if d == "f" else H
        tm_full[seq][B1 * half:B1 * (half + 1), :, lo:lo + H] = tm1

    in_maps2 = []
    for c in range(NCORES):
        sl = slice(PB * c, PB * (c + 1))
        in_maps2.append({
            "tmA": np.ascontiguousarray(tm_full[0][sl]),
            "tmB": np.ascontiguousarray(tm_full[1][sl]),
        })
    if "nc2" not in _CACHE:
        _CACHE["nc2"] = _build_run2()
    res2 = run_bass_kernel_spmd(_CACHE["nc2"], in_maps2,
                                core_ids=list(range(NCORES)))

    outA = np.empty((BSZ, T, 4 * H2), np.float32)
    outB = np.empty((BSZ, T, 4 * H2), np.float32)
    outA[:, :, 0:H2] = tm_full[0]
    outB[:, :, 0:H2] = tm_full[1]
    for c in range(NCORES):
        sl = slice(PB * c, PB * (c + 1))
        outA[sl, :, H2:] = res2.results[c]["oA"]
        outB[sl, :, H2:] = res2.results[c]["oB"]
    return outA, outB
